# revision 15
# baseline (speedup 1.0000x reference)
"""ConvLSTM3D Trainium2 kernel.

Data-parallel over batch: 8 batch elements -> 8 NeuronCores, weights
replicated.  Per core, each timestep computes a 3x3x3 conv over
concat(x_t, H) via PSUM-accumulated bf16 matmuls, then the LSTM gate
math on DVE/ACT in fp32.

Matmul structure per output tile [128 out-ch, 512 cols]:
  - 9 fused K=128 matmuls: HH holds H_pad on partitions 0-63 and the
    one-column-left-shifted copy on 64-127, so one matmul reading the
    kw=-1 window contracts taps kw=-1 AND kw=0 at once.
  - 1 K=128 im2col matmul for the whole x contribution (27 taps
    stacked on partitions).
  - The 9 remaining kw=+1 taps run as K=64 row-tiled matmuls: the PE
    array splits into two 64x128 strips; strip T0 (rows 0-63, lower
    HH window at cols 2:34) computes out-ch 0-127 while strip T8
    (rows 64-127, shifted-copy window at cols 1:33) concurrently
    computes out-ch 128-255 into the other PSUM bank.  Interleaved
    issue makes the two strips stream simultaneously, halving the
    time of the singles phase.

Gate math: i/f packed 128-wide; tanh(c_c) written in-place into the C
buffer so one 128-wide multiply produces [i*tanh(c_c); f*C_old];
partition-crossing hops and PSUM evacuation run on SBUF-SBUF/PSUM-SBUF
DMAs instead of GPSIMD (whose per-op overhead is ~4.5us).
"""
import numpy as np
import ml_dtypes
from contextlib import ExitStack
from itertools import product

import concourse.bacc as bacc
import concourse.bass as bass
import concourse.tile as tile
import concourse.mybir as mybir

f32 = mybir.dt.float32
bf16 = mybir.dt.bfloat16

# Problem constants (hardcoded per harness contract)
B, C_IN, T, D, HS, WS = 8, 4, 8, 16, 32, 32
C_OUT = 64
PD, PH, PW = D + 2, HS + 2, WS + 2          # 18, 34, 34
PLANE = PH * PW                              # 1156
NPAD = PD * PLANE                            # 20808
NSP = D * HS * WS                            # 16384
QOFF = PLANE + PW + 1                        # 1191, min padded read index
XI_ROWS = 543                                # ceil((NPAD - 2*QOFF)/34) aligned
XI_COLS = XI_ROWS * PW                       # 18462

TAPS = [(kd, kh, kw) for kd, kh, kw in product((-1, 0, 1), repeat=3)]  # 27
KDKH = [(kd, kh) for kd, kh in product((-1, 0, 1), repeat=2)]          # 9
N_WSLOT = 19  # 9 fused (kw=-1,0) + 9 strip-pair (kw=+1) + 1 x-im2col


def build_nc(t_steps=T, d_depth=D):
    """Build the Bass program. Reduced (t_steps, d_depth) for sim tests
    still uses the full padded plane geometry, just fewer planes/steps."""
    nsp = d_depth * HS * WS
    npad = (d_depth + 2) * PLANE
    plane = PLANE
    # X im2col is split into two half-depth buffers so each half's refill
    # for step t+1 overlaps the other half's compute (chunk d reads rows
    # 34d..34d+32, so the split at chunk dsplit is clean).
    dsplit = (d_depth + 1) // 2
    rows_a = 34 * (dsplit - 1) + 32          # rows 0..rows_a of XI_A
    cols_a = rows_a * PW
    rowb0 = 34 * dsplit                      # first global row of XI_B
    cols_b = (34 * (d_depth - 1) + 32 - rowb0) * PW
    offb = rowb0 * PW

    nc = bacc.Bacc("TRN2", target_bir_lowering=False, debug=False)

    xpad_h = nc.dram_tensor("xpad", [C_IN, t_steps, npad], bf16, kind="ExternalInput")
    whwx_h = nc.dram_tensor("whwx", [128, N_WSLOT * 256], bf16, kind="ExternalInput")
    bias_h = nc.dram_tensor("bias", [128, 4], f32, kind="ExternalInput")
    ppif_h = nc.dram_tensor("ppif", [128, nsp], bf16, kind="ExternalInput")
    ppo_h = nc.dram_tensor("ppo", [64, nsp], bf16, kind="ExternalInput")
    y_h = nc.dram_tensor("y", [C_OUT, t_steps * nsp], f32, kind="ExternalOutput")

    with tile.TileContext(nc) as tc, ExitStack() as ctx:
        const = ctx.enter_context(tc.tile_pool(name="const", bufs=1))
        pp_pool = ctx.enter_context(tc.tile_pool(name="pp", bufs=2))
        sc = ctx.enter_context(tc.tile_pool(name="sc", bufs=2))
        mosc = ctx.enter_context(tc.tile_pool(name="mosc", bufs=2))
        psum = ctx.enter_context(tc.tile_pool(name="psum", bufs=2, space="PSUM"))

        HH = const.tile([128, npad], bf16)
        XIA = const.tile([128, cols_a], bf16)
        XIB = const.tile([128, cols_b], bf16)
        C2 = const.tile([128, nsp], f32)
        WW = const.tile([128, N_WSLOT * 256], bf16)
        BIAS = const.tile([128, 4], f32)

        nc.vector.memset(HH[:], 0.0)
        nc.vector.memset(XIA[:], 0.0)
        nc.vector.memset(XIB[:], 0.0)
        nc.vector.memset(C2[:], 0.0)
        nc.sync.dma_start(WW[:], whwx_h[:])
        nc.sync.dma_start(BIAS[:], bias_h[:])

        hh3 = HH[:].rearrange("p (r c) -> p r c", c=PW)
        xia3 = XIA[:].rearrange("p (r c) -> p r c", c=PW)
        xib3 = XIB[:].rearrange("p (r c) -> p r c", c=PW)
        ww3 = WW[:].rearrange("p (s m) -> p s m", m=256)

        def fill_xi(xt, half):
            """Refill one im2col half with step-xt x data (27 tap copies)."""
            xi_t, base, cols = (XIA, 0, cols_a) if half == 0 else (XIB, offb, cols_b)
            for j, (kd, kh, kw) in enumerate(TAPS):
                oj = kd * plane + kh * PW + kw
                ln = min(cols, npad - QOFF - oj - base)
                nc.sync.dma_start(
                    xi_t[4 * j:4 * j + 4, 0:ln],
                    xpad_h[0:4, xt, QOFF + oj + base: QOFF + oj + base + ln],
                )

        b_if = BIAS[:, 0:1]
        b_c = BIAS[0:64, 1:2]
        b_o = BIAS[64:128, 2:3]       # o-bias hosted on upper partitions
        b_zero = BIAS[64:128, 3:4]    # host-provided zeros (avoid const_aps)

        def emit_mm(t, d):
            """Conv matmuls for chunk (t, d) -> (p0, p1) PSUM tiles."""
            p0 = psum.tile([128, 1024], f32, tag="p0")
            p1 = psum.tile([128, 1024], f32, tag="p1")
            # full-array phase: 9 fused (t>0) + 1 im2col per (mt, hb)
            for mt, ptile in ((0, p0), (1, p1)):
                msl = slice(mt * 128, (mt + 1) * 128)
                for hb in (0, 1):
                    out_ap = ptile[:, hb * 512:(hb + 1) * 512]
                    first = True
                    if t > 0:
                        for s, (kd, kh) in enumerate(KDKH):
                            r0 = (d + 1 + kd) * 34 + hb * 16 + 1 + kh
                            # fused: lower=tap kw=-1, upper(shifted)=tap kw=0
                            nc.tensor.matmul(
                                out_ap, ww3[:, s, msl],
                                hh3[:, r0:r0 + 16, 0:32],
                                start=first, stop=False, skip_group_check=True,
                            )
                            first = False
                    if d < dsplit:
                        xi3 = xia3
                        r0x = 34 * d + hb * 16
                    else:
                        xi3 = xib3
                        r0x = 34 * (d - dsplit) + hb * 16
                    nc.tensor.matmul(
                        out_ap, ww3[:, 18, msl],
                        xi3[:, r0x:r0x + 16, 0:32],
                        start=first, stop=(t == 0), skip_group_check=True,
                    )
            # strip phase (t>0): kw=+1 taps as row-tiled K=64 matmuls.
            # T0 (PE rows 0-63) accumulates out-ch 0-127 into p0 while T8
            # (rows 64-127, reading the pre-shifted upper HH copy at a
            # one-left window) concurrently accumulates out-ch 128-255
            # into p1 -- different PSUM banks, so the strips overlap.
            if t > 0:
                for hb in (0, 1):
                    for s, (kd, kh) in enumerate(KDKH):
                        r0 = (d + 1 + kd) * 34 + hb * 16 + 1 + kh
                        last = s == 8
                        nc.tensor.matmul(
                            p0[:, hb * 512:(hb + 1) * 512],
                            ww3[0:64, 9 + s, 0:128],
                            hh3[0:64, r0:r0 + 16, 2:34],
                            start=False, stop=last, skip_group_check=True,
                        )
                        nc.tensor.matmul(
                            p1[:, hb * 512:(hb + 1) * 512],
                            ww3[64:128, 9 + s, 128:256],
                            hh3[64:128, r0:r0 + 16, 1:33],
                            start=False, stop=last, skip_group_check=True,
                        )
            return p0, p1

        def emit_el(t, d, p0, p1):
            """Gate math + state/output writes for chunk (t, d).

            The i/f/Cn path runs on partitions 0-63 (plus the 128-wide
            packed ops); the o/Hn path runs on partitions 64-127, where
            the duplicated Cn already lives -- this lets the o_c PSUM
            half evacuate early via an in-partition ACT copy."""
            csl = slice(d * 1024, (d + 1) * 1024)
            ppo_t = pp_pool.tile([128, 1024], bf16, tag="ppo")
            nc.sync.dma_start(ppo_t[64:128, :], ppo_h[:, csl])
            hrow = (d + 1) * 34 + 1
            oc = mosc.tile([128, 1024], f32, tag="oc")

            if t == 0:
                # H=0, C=0: i_g=sig(i_c), Cn=i_g*tanh(c_c), o=sig(o_c+W_co*Cn)
                nc.scalar.activation(oc[64:128, :], p1[64:128, :],
                                     mybir.ActivationFunctionType.Copy)  # evac o_c
                nc.scalar.activation(C2[0:64, csl], p1[0:64, :],
                                     mybir.ActivationFunctionType.Tanh,
                                     bias=b_c)                    # tanh(c_c)
                g = sc.tile([128, 1024], f32, tag="mf")
                nc.scalar.activation(g[0:64, :], p0[0:64, :],
                                     mybir.ActivationFunctionType.Sigmoid,
                                     bias=BIAS[0:64, 0:1])        # i_g
                nc.vector.tensor_mul(C2[0:64, csl], C2[0:64, csl], g[0:64, :])  # Cn
            else:
                ppif_t = pp_pool.tile([128, 1024], bf16, tag="ppif")
                nc.sync.dma_start(ppif_t[:], ppif_h[:, csl])
                mf2 = sc.tile([128, 1024], f32, tag="mf")
                nc.vector.tensor_mul(mf2[:], ppif_t[:], C2[:, csl])   # W_cif*C
                nc.vector.tensor_add(mf2[:], mf2[:], p0[:])           # + conv_if
                nc.scalar.activation(oc[64:128, :], p1[64:128, :],
                                     mybir.ActivationFunctionType.Copy)  # evac o_c
                # tanh(c_c) lands in-place in the C buffer's lower half
                # (old C survives in the upper half for the f-gate term)
                nc.scalar.activation(C2[0:64, csl], p1[0:64, :],
                                     mybir.ActivationFunctionType.Tanh,
                                     bias=b_c)
                nc.scalar.activation(mf2[:], mf2[:],
                                     mybir.ActivationFunctionType.Sigmoid,
                                     bias=b_if)                       # [i_g ; f_g]
                vw = sc.tile([128, 1024], f32, tag="vw")
                nc.vector.tensor_mul(vw[:], mf2[:], C2[:, csl])  # [i*tc ; f*C]
                w0 = mosc.tile([128, 1024], f32, tag="wh")
                nc.vector.tensor_copy(w0[0:64, :], vw[64:128, :])  # partition hop
                nc.vector.tensor_add(C2[0:64, csl], vw[0:64, :], w0[0:64, :])  # Cn

            nc.gpsimd.tensor_copy(C2[64:128, csl], C2[0:64, csl])  # dup Cn
            th = mosc.tile([128, 1024], f32, tag="th")
            nc.scalar.activation(th[64:128, :], C2[64:128, csl],
                                 mybir.ActivationFunctionType.Tanh,
                                 bias=b_zero)                     # tanh(Cn)
            mo = mosc.tile([128, 1024], f32, tag="mo")
            nc.vector.tensor_mul(mo[64:128, :], ppo_t[64:128, :],
                                 C2[64:128, csl])                 # W_co*Cn
            nc.vector.tensor_add(mo[64:128, :], mo[64:128, :], oc[64:128, :])
            nc.scalar.activation(mo[64:128, :], mo[64:128, :],
                                 mybir.ActivationFunctionType.Sigmoid,
                                 bias=b_o)                        # o_g in place
            hf = mosc.tile([128, 1024], f32, tag="wh")
            nc.vector.tensor_mul(hf[64:128, :], mo[64:128, :], th[64:128, :])
            # write-backs: ACT writes the (pre-shifted) upper bf16 H copy,
            # a cross-base DVE copy derives the lower copy, DMA writes fp32
            hf3 = hf[64:128, :].rearrange("p (r c) -> p r c", c=32)
            nc.scalar.activation(hh3[64:128, hrow:hrow + 32, 0:32], hf3,
                                 mybir.ActivationFunctionType.Copy)
            nc.vector.tensor_copy(hh3[0:64, hrow:hrow + 32, 1:33], hf3)
            nc.sync.dma_start(y_h[:, t * nsp + d * 1024: t * nsp + (d + 1) * 1024],
                              hf[64:128, :])

        fill_xi(0, 0)
        fill_xi(0, 1)
        for t in range(t_steps):
            # Chunk d's conv reads H planes d-1..d+1 of the *previous* step,
            # but emit_el(d) overwrites plane d in place.  Emitting el(d-1)
            # after mm(d) makes Tile's WAR deps order every read of plane
            # d-1 before its overwrite (one-chunk software pipeline).
            # Each im2col half refills with step-t+1 data right after its
            # last reader of step t, hiding the fill under compute.
            prev = None
            for d in range(d_depth):
                cur = emit_mm(t, d)
                if prev is not None:
                    emit_el(t, d - 1, *prev)
                if d == dsplit - 1 and t + 1 < t_steps:
                    fill_xi(t + 1, 0)
                prev = cur
            emit_el(t, d_depth - 1, *prev)
            if t + 1 < t_steps:
                fill_xi(t + 1, 1)

    nc.finalize()
    return nc


# ---------------------------------------------------------------------------
# host-side input prep

def prep_weights(Wc, b, W_ci, W_cf, W_co):
    Wc = np.asarray(Wc, np.float32)
    wh = np.zeros((128, N_WSLOT, 256), np.float32)
    for s, (kd, kh) in enumerate(KDKH):
        # fused slot: lower rows = tap kw=-1, upper rows = tap kw=0
        wh[0:64, s, :] = Wc[:, 4:68, kd + 1, kh + 1, 0].T
        wh[64:128, s, :] = Wc[:, 4:68, kd + 1, kh + 1, 1].T
        # strip-pair slot for tap kw=+1: T0 (rows 0-63) holds out-ch
        # 0-127, T8 (rows 64-127) holds out-ch 128-255
        wh[0:64, 9 + s, 0:128] = Wc[0:128, 4:68, kd + 1, kh + 1, 2].T
        wh[64:128, 9 + s, 128:256] = Wc[128:256, 4:68, kd + 1, kh + 1, 2].T
    for j, (kd, kh, kw) in enumerate(TAPS):
        for c in range(C_IN):
            wh[4 * j + c, 18, :] = Wc[:, c, kd + 1, kh + 1, kw + 1]
    whwx = wh.reshape(128, N_WSLOT * 256).astype(ml_dtypes.bfloat16)

    bias = np.zeros((128, 4), np.float32)
    b = np.asarray(b, np.float32)
    bias[:, 0] = b[0:128]                      # i ; f
    bias[0:64, 1] = b[128:192]                 # c
    bias[64:128, 2] = b[192:256]               # o (upper partitions: o-path
    #                                            runs on partitions 64-127)

    ppif = np.concatenate([
        np.asarray(W_ci, np.float32).reshape(64, NSP),
        np.asarray(W_cf, np.float32).reshape(64, NSP),
    ], axis=0).astype(ml_dtypes.bfloat16)
    ppo = np.asarray(W_co, np.float32).reshape(64, NSP).astype(ml_dtypes.bfloat16)
    return whwx, bias, ppif, ppo


def prep_x(Xb):
    """[C_IN, T, D, H, W] fp32 -> padded bf16 [C_IN, T, NPAD]."""
    xp = np.zeros((C_IN, T, PD, PH, PW), np.float32)
    xp[:, :, 1:1 + D, 1:1 + HS, 1:1 + WS] = Xb
    return xp.reshape(C_IN, T, NPAD).astype(ml_dtypes.bfloat16)


_NC_CACHE = {}
_LAST_RESULTS = {}


def _get_nc():
    if "nc" not in _NC_CACHE:
        _NC_CACHE["nc"] = build_nc()
    return _NC_CACHE["nc"]


def kernel(X, Wc, b, W_ci, W_cf, W_co):
    import os
    from concourse.bass_utils import run_bass_kernel_spmd

    X = np.asarray(X, np.float32)
    whwx, bias, ppif, ppo = prep_weights(Wc, b, W_ci, W_cf, W_co)
    in_maps = []
    for bi in range(B):
        in_maps.append({
            "xpad": prep_x(X[bi]),
            "whwx": whwx,
            "bias": bias,
            "ppif": ppif,
            "ppo": ppo,
        })
    nc = _get_nc()
    trace = os.environ.get("TRACE_BASS", "0") == "1"
    res = run_bass_kernel_spmd(nc, in_maps, core_ids=list(range(B)), trace=trace)
    _LAST_RESULTS["br"] = res
    out = np.stack([
        np.asarray(res.results[bi]["y"]).reshape(C_OUT, T, D, HS, WS)
        for bi in range(B)
    ], axis=0)
    return out.astype(np.float32)


# revision 18
# speedup vs baseline: 1.1325x; 1.1325x over previous
"""ConvLSTM3D Trainium2 kernel.

Data-parallel over batch: 8 batch elements -> 8 NeuronCores, weights
replicated.  Per core, each timestep computes a 3x3x3 conv over
concat(x_t, H) via PSUM-accumulated bf16 matmuls, then the LSTM gate
math on DVE/ACT in fp32.

Matmul structure per output tile [128 out-ch, 512 cols]:
  - 9 fused K=128 matmuls: HH holds H_pad on partitions 0-63 and the
    one-column-left-shifted copy on 64-127, so one matmul reading the
    kw=-1 window contracts taps kw=-1 AND kw=0 at once.
  - 1 K=128 im2col matmul for the whole x contribution (27 taps
    stacked on partitions).
  - The 9 remaining kw=+1 taps run as K=64 row-tiled matmuls: the PE
    array splits into two 64x128 strips; strip T0 (rows 0-63, lower
    HH window at cols 2:34) computes out-ch 0-127 while strip T8
    (rows 64-127, shifted-copy window at cols 1:33) concurrently
    computes out-ch 128-255 into the other PSUM bank.  Interleaved
    issue makes the two strips stream simultaneously, halving the
    time of the singles phase.

Gate math: i/f packed 128-wide; tanh(c_c) written in-place into the C
buffer so one 128-wide multiply produces [i*tanh(c_c); f*C_old];
partition-crossing hops and PSUM evacuation run on SBUF-SBUF/PSUM-SBUF
DMAs instead of GPSIMD (whose per-op overhead is ~4.5us).
"""
import numpy as np
import ml_dtypes
from contextlib import ExitStack
from itertools import product

import concourse.bacc as bacc
import concourse.bass as bass
import concourse.tile as tile
import concourse.mybir as mybir

f32 = mybir.dt.float32
bf16 = mybir.dt.bfloat16

# Problem constants (hardcoded per harness contract)
B, C_IN, T, D, HS, WS = 8, 4, 8, 16, 32, 32
C_OUT = 64
PD, PH, PW = D + 2, HS + 2, WS + 2          # 18, 34, 34
PLANE = PH * PW                              # 1156
NPAD = PD * PLANE                            # 20808
NSP = D * HS * WS                            # 16384
QOFF = PLANE + PW + 1                        # 1191, min padded read index
XI_ROWS = 543                                # ceil((NPAD - 2*QOFF)/34) aligned
XI_COLS = XI_ROWS * PW                       # 18462

TAPS = [(kd, kh, kw) for kd, kh, kw in product((-1, 0, 1), repeat=3)]  # 27
KDKH = [(kd, kh) for kd, kh in product((-1, 0, 1), repeat=2)]          # 9
N_WSLOT = 19  # 9 fused (kw=-1,0) + 9 strip-pair (kw=+1) + 1 x-im2col


def build_nc(t_steps=T, d_depth=D):
    """Build the Bass program. Reduced (t_steps, d_depth) for sim tests
    still uses the full padded plane geometry, just fewer planes/steps."""
    nsp = d_depth * HS * WS
    npad = (d_depth + 2) * PLANE
    plane = PLANE
    # X im2col is split into two half-depth buffers so each half's refill
    # for step t+1 overlaps the other half's compute (chunk d reads rows
    # 34d..34d+32, so the split at chunk dsplit is clean).
    dsplit = (d_depth + 1) // 2
    rows_a = 34 * (dsplit - 1) + 32          # rows 0..rows_a of XI_A
    cols_a = rows_a * PW
    rowb0 = 34 * dsplit                      # first global row of XI_B
    cols_b = (34 * (d_depth - 1) + 32 - rowb0) * PW
    offb = rowb0 * PW

    nc = bacc.Bacc("TRN2", target_bir_lowering=False, debug=False)

    xpad_h = nc.dram_tensor("xpad", [C_IN, t_steps, npad], bf16, kind="ExternalInput")
    whwx_h = nc.dram_tensor("whwx", [128, N_WSLOT * 256], bf16, kind="ExternalInput")
    bias_h = nc.dram_tensor("bias", [128, 4], f32, kind="ExternalInput")
    ppif_h = nc.dram_tensor("ppif", [128, nsp], f32, kind="ExternalInput")
    ppo_h = nc.dram_tensor("ppo", [64, nsp], bf16, kind="ExternalInput")
    y_h = nc.dram_tensor("y", [C_OUT, t_steps * nsp], f32, kind="ExternalOutput")

    with tile.TileContext(nc) as tc, ExitStack() as ctx:
        const = ctx.enter_context(tc.tile_pool(name="const", bufs=1))
        pp_pool = ctx.enter_context(tc.tile_pool(name="pp", bufs=2))
        sc = ctx.enter_context(tc.tile_pool(name="sc", bufs=2))
        mosc = ctx.enter_context(tc.tile_pool(name="mosc", bufs=2))
        psum = ctx.enter_context(tc.tile_pool(name="psum", bufs=2, space="PSUM"))

        HH = const.tile([128, npad], bf16)
        XIA = const.tile([128, cols_a], bf16)
        XIB = const.tile([128, cols_b], bf16)
        C2 = const.tile([128, nsp], f32)
        WW = const.tile([128, N_WSLOT * 256], bf16)
        BIAS = const.tile([128, 4], f32)

        nc.vector.memset(HH[:], 0.0)
        nc.vector.memset(XIA[:], 0.0)
        nc.vector.memset(XIB[:], 0.0)
        nc.vector.memset(C2[:], 0.0)
        nc.sync.dma_start(WW[:], whwx_h[:])
        nc.sync.dma_start(BIAS[:], bias_h[:])

        hh3 = HH[:].rearrange("p (r c) -> p r c", c=PW)
        xia3 = XIA[:].rearrange("p (r c) -> p r c", c=PW)
        xib3 = XIB[:].rearrange("p (r c) -> p r c", c=PW)
        ww3 = WW[:].rearrange("p (s m) -> p s m", m=256)

        def fill_xi(xt, half):
            """Refill one im2col half with step-xt x data (27 tap copies)."""
            xi_t, base, cols = (XIA, 0, cols_a) if half == 0 else (XIB, offb, cols_b)
            for j, (kd, kh, kw) in enumerate(TAPS):
                oj = kd * plane + kh * PW + kw
                ln = min(cols, npad - QOFF - oj - base)
                nc.sync.dma_start(
                    xi_t[4 * j:4 * j + 4, 0:ln],
                    xpad_h[0:4, xt, QOFF + oj + base: QOFF + oj + base + ln],
                )

        b_if = BIAS[:, 0:1]
        b_c = BIAS[0:64, 1:2]
        b_o = BIAS[64:128, 2:3]       # o-bias hosted on upper partitions
        b_zero = BIAS[64:128, 3:4]    # host-provided zeros (avoid const_aps)

        def emit_mm(t, d):
            """Conv matmuls for chunk (t, d) -> (p0, p1) PSUM tiles."""
            p0 = psum.tile([128, 1024], f32, tag="p0")
            p1 = psum.tile([128, 1024], f32, tag="p1")
            # full-array phase: 9 fused (t>0) + 1 im2col per (mt, hb)
            for mt, ptile in ((0, p0), (1, p1)):
                msl = slice(mt * 128, (mt + 1) * 128)
                for hb in (0, 1):
                    out_ap = ptile[:, hb * 512:(hb + 1) * 512]
                    first = True
                    if t > 0:
                        for s, (kd, kh) in enumerate(KDKH):
                            r0 = (d + 1 + kd) * 34 + hb * 16 + 1 + kh
                            # fused: lower=tap kw=-1, upper(shifted)=tap kw=0
                            nc.tensor.matmul(
                                out_ap, ww3[:, s, msl],
                                hh3[:, r0:r0 + 16, 0:32],
                                start=first, stop=False, skip_group_check=True,
                            )
                            first = False
                    if d < dsplit:
                        xi3 = xia3
                        r0x = 34 * d + hb * 16
                    else:
                        xi3 = xib3
                        r0x = 34 * (d - dsplit) + hb * 16
                    nc.tensor.matmul(
                        out_ap, ww3[:, 18, msl],
                        xi3[:, r0x:r0x + 16, 0:32],
                        start=first, stop=(t == 0), skip_group_check=True,
                    )
            # strip phase (t>0): kw=+1 taps as row-tiled K=64 matmuls.
            # T0 (PE rows 0-63) accumulates out-ch 0-127 into p0 while T8
            # (rows 64-127, reading the pre-shifted upper HH copy at a
            # one-left window) concurrently accumulates out-ch 128-255
            # into p1 -- different PSUM banks, so the strips overlap.
            if t > 0:
                for hb in (0, 1):
                    for s, (kd, kh) in enumerate(KDKH):
                        r0 = (d + 1 + kd) * 34 + hb * 16 + 1 + kh
                        last = s == 8
                        nc.tensor.matmul(
                            p0[:, hb * 512:(hb + 1) * 512],
                            ww3[0:64, 9 + s, 0:128],
                            hh3[0:64, r0:r0 + 16, 2:34],
                            start=False, stop=last, skip_group_check=True,
                        )
                        nc.tensor.matmul(
                            p1[:, hb * 512:(hb + 1) * 512],
                            ww3[64:128, 9 + s, 128:256],
                            hh3[64:128, r0:r0 + 16, 1:33],
                            start=False, stop=last, skip_group_check=True,
                        )
            return p0, p1

        def emit_el(t, d, p0, p1):
            """Gate math + state/output writes for chunk (t, d).

            The i/f/Cn path runs on partitions 0-63 (plus the 128-wide
            packed ops); the o/Hn path runs on partitions 64-127, where
            the duplicated Cn already lives -- this lets the o_c PSUM
            half evacuate early via an in-partition ACT copy."""
            csl = slice(d * 1024, (d + 1) * 1024)
            ppo_t = pp_pool.tile([128, 1024], bf16, tag="ppo")
            nc.sync.dma_start(ppo_t[64:128, :], ppo_h[:, csl])
            hrow = (d + 1) * 34 + 1
            oc = mosc.tile([128, 1024], f32, tag="oc")

            if t == 0:
                # H=0, C=0: i_g=sig(i_c), Cn=i_g*tanh(c_c), o=sig(o_c+W_co*Cn)
                nc.scalar.activation(oc[64:128, :], p1[64:128, :],
                                     mybir.ActivationFunctionType.Copy)  # evac o_c
                nc.scalar.activation(C2[0:64, csl], p1[0:64, :],
                                     mybir.ActivationFunctionType.Tanh,
                                     bias=b_c)                    # tanh(c_c)
                g = sc.tile([128, 1024], f32, tag="mf")
                nc.scalar.activation(g[0:64, :], p0[0:64, :],
                                     mybir.ActivationFunctionType.Sigmoid,
                                     bias=BIAS[0:64, 0:1])        # i_g
                nc.vector.tensor_mul(C2[0:64, csl], C2[0:64, csl], g[0:64, :])  # Cn
            else:
                ppif_t = pp_pool.tile([128, 1024], f32, tag="ppif")
                nc.sync.dma_start(ppif_t[:], ppif_h[:, csl])
                mf2 = sc.tile([128, 1024], f32, tag="mf")
                nc.gpsimd.tensor_mul(mf2[:], ppif_t[:], C2[:, csl])   # W_cif*C
                nc.vector.tensor_add(mf2[:], mf2[:], p0[:])           # + conv_if
                nc.scalar.activation(oc[64:128, :], p1[64:128, :],
                                     mybir.ActivationFunctionType.Copy)  # evac o_c
                # tanh(c_c) lands in-place in the C buffer's lower half
                # (old C survives in the upper half for the f-gate term)
                nc.scalar.activation(C2[0:64, csl], p1[0:64, :],
                                     mybir.ActivationFunctionType.Tanh,
                                     bias=b_c)
                nc.scalar.activation(mf2[:], mf2[:],
                                     mybir.ActivationFunctionType.Sigmoid,
                                     bias=b_if)                       # [i_g ; f_g]
                vw = sc.tile([128, 1024], f32, tag="vw")
                nc.vector.tensor_mul(vw[:], mf2[:], C2[:, csl])  # [i*tc ; f*C]
                w0 = mosc.tile([128, 1024], f32, tag="wh")
                nc.vector.tensor_copy(w0[0:64, :], vw[64:128, :])  # partition hop
                nc.vector.tensor_add(C2[0:64, csl], vw[0:64, :], w0[0:64, :])  # Cn

            nc.vector.tensor_copy(C2[64:128, csl], C2[0:64, csl])  # dup Cn
            th = mosc.tile([128, 1024], bf16, tag="th")
            nc.scalar.activation(th[64:128, :], C2[64:128, csl],
                                 mybir.ActivationFunctionType.Tanh,
                                 bias=b_zero)                     # tanh(Cn)
            mo = mosc.tile([128, 1024], f32, tag="mo")
            nc.vector.tensor_mul(mo[64:128, :], ppo_t[64:128, :],
                                 C2[64:128, csl])                 # W_co*Cn
            nc.vector.tensor_add(mo[64:128, :], mo[64:128, :], oc[64:128, :])
            nc.scalar.activation(mo[64:128, :], mo[64:128, :],
                                 mybir.ActivationFunctionType.Sigmoid,
                                 bias=b_o)                        # o_g in place
            hf = mosc.tile([128, 1024], f32, tag="wh")
            nc.vector.tensor_mul(hf[64:128, :], mo[64:128, :], th[64:128, :])
            # write-backs: ACT writes the (pre-shifted) upper bf16 H copy,
            # a cross-base DVE copy derives the lower copy, DMA writes fp32
            hf3 = hf[64:128, :].rearrange("p (r c) -> p r c", c=32)
            nc.scalar.activation(hh3[64:128, hrow:hrow + 32, 0:32], hf3,
                                 mybir.ActivationFunctionType.Copy)
            nc.vector.tensor_copy(hh3[0:64, hrow:hrow + 32, 1:33], hf3)
            nc.sync.dma_start(y_h[:, t * nsp + d * 1024: t * nsp + (d + 1) * 1024],
                              hf[64:128, :])

        fill_xi(0, 0)
        fill_xi(0, 1)
        for t in range(t_steps):
            # Chunk d's conv reads H planes d-1..d+1 of the *previous* step,
            # but emit_el(d) overwrites plane d in place.  Emitting el(d-1)
            # after mm(d) makes Tile's WAR deps order every read of plane
            # d-1 before its overwrite (one-chunk software pipeline).
            # Each im2col half refills with step-t+1 data right after its
            # last reader of step t, hiding the fill under compute.
            prev = None
            for d in range(d_depth):
                cur = emit_mm(t, d)
                if prev is not None:
                    emit_el(t, d - 1, *prev)
                if d == dsplit - 1 and t + 1 < t_steps:
                    fill_xi(t + 1, 0)
                prev = cur
            emit_el(t, d_depth - 1, *prev)
            if t + 1 < t_steps:
                fill_xi(t + 1, 1)

    nc.finalize()
    return nc


# ---------------------------------------------------------------------------
# host-side input prep

def prep_weights(Wc, b, W_ci, W_cf, W_co):
    Wc = np.asarray(Wc, np.float32)
    wh = np.zeros((128, N_WSLOT, 256), np.float32)
    for s, (kd, kh) in enumerate(KDKH):
        # fused slot: lower rows = tap kw=-1, upper rows = tap kw=0
        wh[0:64, s, :] = Wc[:, 4:68, kd + 1, kh + 1, 0].T
        wh[64:128, s, :] = Wc[:, 4:68, kd + 1, kh + 1, 1].T
        # strip-pair slot for tap kw=+1: T0 (rows 0-63) holds out-ch
        # 0-127, T8 (rows 64-127) holds out-ch 128-255
        wh[0:64, 9 + s, 0:128] = Wc[0:128, 4:68, kd + 1, kh + 1, 2].T
        wh[64:128, 9 + s, 128:256] = Wc[128:256, 4:68, kd + 1, kh + 1, 2].T
    for j, (kd, kh, kw) in enumerate(TAPS):
        for c in range(C_IN):
            wh[4 * j + c, 18, :] = Wc[:, c, kd + 1, kh + 1, kw + 1]
    whwx = wh.reshape(128, N_WSLOT * 256).astype(ml_dtypes.bfloat16)

    bias = np.zeros((128, 4), np.float32)
    b = np.asarray(b, np.float32)
    bias[:, 0] = b[0:128]                      # i ; f
    bias[0:64, 1] = b[128:192]                 # c
    bias[64:128, 2] = b[192:256]               # o (upper partitions: o-path
    #                                            runs on partitions 64-127)

    ppif = np.concatenate([
        np.asarray(W_ci, np.float32).reshape(64, NSP),
        np.asarray(W_cf, np.float32).reshape(64, NSP),
    ], axis=0)
    ppo = np.asarray(W_co, np.float32).reshape(64, NSP).astype(ml_dtypes.bfloat16)
    return whwx, bias, ppif, ppo


def prep_x(Xb):
    """[C_IN, T, D, H, W] fp32 -> padded bf16 [C_IN, T, NPAD]."""
    xp = np.zeros((C_IN, T, PD, PH, PW), np.float32)
    xp[:, :, 1:1 + D, 1:1 + HS, 1:1 + WS] = Xb
    return xp.reshape(C_IN, T, NPAD).astype(ml_dtypes.bfloat16)


_NC_CACHE = {}
_LAST_RESULTS = {}


def _get_nc():
    if "nc" not in _NC_CACHE:
        _NC_CACHE["nc"] = build_nc()
    return _NC_CACHE["nc"]


def kernel(X, Wc, b, W_ci, W_cf, W_co):
    import os
    from concourse.bass_utils import run_bass_kernel_spmd

    X = np.asarray(X, np.float32)
    whwx, bias, ppif, ppo = prep_weights(Wc, b, W_ci, W_cf, W_co)
    in_maps = []
    for bi in range(B):
        in_maps.append({
            "xpad": prep_x(X[bi]),
            "whwx": whwx,
            "bias": bias,
            "ppif": ppif,
            "ppo": ppo,
        })
    nc = _get_nc()
    trace = os.environ.get("TRACE_BASS", "0") == "1"
    res = run_bass_kernel_spmd(nc, in_maps, core_ids=list(range(B)), trace=trace)
    _LAST_RESULTS["br"] = res
    out = np.stack([
        np.asarray(res.results[bi]["y"]).reshape(C_OUT, T, D, HS, WS)
        for bi in range(B)
    ], axis=0)
    return out.astype(np.float32)
